# revision 3
# baseline (speedup 1.0000x reference)
import numpy as np
import jax
import jax.numpy as jnp
from functools import partial

# KPConv regressor: N=50000 points, NN=32 neighbors, K=15 kernel points,
# D_IN=64, D_OUT=1024, B=16 graphs, head 1024->512->256->152.
# Data-parallel over points across 8 NeuronCores; pos/feats tables +
# kernel weights replicated; per-core partial pooled sums combined at end.
SIGMA = 0.3
B = 16
N = 50000
NC = 8
PAD_N = 50048  # next multiple of 8*128 above 50000
CHUNK = PAD_N // NC


@partial(jax.pmap, axis_name="i", in_axes=(0, 0, 0, None, None, None, None))
def _kpconv_shard(pos_c, idx_c, onehot, pos_full, feats_full, kernel_points,
                  kp_w2):
    # pos_c [C,3]; idx_c [C,NN]; onehot [C,B]; pos_full [PAD_N,3]
    # feats_full [N,D]; kernel_points [K,3]; kp_w2 [K*D, O] bf16
    nbr_pos = pos_full[idx_c]                                           # [C,NN,3]
    nbr_f = feats_full[idx_c]                                           # [C,NN,D]
    rel = nbr_pos - pos_c[:, None, :]                                   # [C,NN,3]
    d2 = jnp.sum((rel[:, :, None, :] - kernel_points[None, None]) ** 2, axis=-1)
    d = jnp.sqrt(d2)
    h = jnp.maximum(0.0, 1.0 - d / SIGMA)                               # [C,NN,K]
    g = jnp.einsum("njk,njd->nkd", h.astype(jnp.bfloat16),
                   nbr_f.astype(jnp.bfloat16),
                   preferred_element_type=jnp.float32)                  # [C,K,D]
    gg = g.reshape(g.shape[0], -1).astype(jnp.bfloat16)                 # [C,K*D]
    x = gg @ kp_w2                                                      # [C,O] f32
    x = jnp.where(x > 0, x, 0.1 * x)                                    # leaky relu
    # masked segment-sum via one-hot matmul (pad rows have zero one-hot)
    pooled = onehot.T.astype(jnp.bfloat16) @ x.astype(jnp.bfloat16)    # [B,O]
    return pooled.astype(jnp.float32)


@jax.jit
def _head(pooled_parts, counts, w1, b1, w2, b2, w3, b3):
    pooled = pooled_parts.sum(0) / jnp.maximum(counts, 1.0)
    h1 = jax.nn.relu(pooled @ w1 + b1)
    h2 = jax.nn.relu(h1 @ w2 + b2)
    return h2 @ w3 + b3


def kernel(pos, feats, kernel_points, kp_weights, w1, b1, w2, b2, w3, b3,
           neighbor_idx, batch):
    D = feats.shape[1]
    K = kernel_points.shape[0]
    O = kp_weights.shape[2]

    # ---- host-side shard prep (layout only) ----
    idx_pad = np.zeros((PAD_N, neighbor_idx.shape[1]), np.int32)
    idx_pad[:N] = neighbor_idx
    pos_pad = np.zeros((PAD_N, 3), np.float32)
    pos_pad[:N] = pos
    onehot = np.zeros((PAD_N, B), np.float32)
    onehot[np.arange(N), batch] = 1.0

    idx_s = idx_pad.reshape(NC, CHUNK, -1)
    pos_s = pos_pad.reshape(NC, CHUNK, 3)
    oh_s = onehot.reshape(NC, CHUNK, B)

    w2f = np.asarray(kp_weights.reshape(K * D, O), np.float32)

    pooled_parts = _kpconv_shard(
        jnp.asarray(pos_s), jnp.asarray(idx_s), jnp.asarray(oh_s),
        jnp.asarray(pos_pad), jnp.asarray(feats),
        jnp.asarray(kernel_points), jnp.asarray(w2f, jnp.bfloat16))

    counts = np.bincount(batch, minlength=B).astype(np.float32)[:, None]
    out = _head(pooled_parts, jnp.asarray(counts), jnp.asarray(w1),
                jnp.asarray(b1), jnp.asarray(w2), jnp.asarray(b2),
                jnp.asarray(w3), jnp.asarray(b3))
    return np.asarray(out, dtype=np.float32)
